# revision 2
# baseline (speedup 1.0000x reference)
"""Deformable conv2d (torchvision semantics: stride=1, pad=0, dil=1,
offset_groups=1, no mask/bias) on 8 TRN2 NeuronCores via Bass/Tile.

Hardcoded shapes: input [16,64,66,66] f32, offset [16,18,64,64] f32,
weight [64,64,3,3] f32 -> out [16,64,64,64] f32.

Sharding: data-parallel over batch; core i handles images (2i, 2i+1).

Per-core scheme, partitions p = (im in 2, c in 64):
  - value table tab[p, (ty*68+tx)*4 + j] = I[im][c][ty-1+jy, tx-1+jx]
    (j=2*jy+jx, zero border): one d=4 ap_gather per (tap, 2048-px chunk)
    fetches all 4 bilinear corners of a pixel; all 16-partition groups of
    an image share one index stream (idx wrapped by 16).
  - per-pixel bilinear corner weights (low-side validity folded in;
    high-side OOB handled by zero table cells) are computed compactly on
    DVE ([128,576] cells), dumped to DRAM, then replicated across
    partitions by stride-0-source DMA broadcasts (2 per iteration,
    one per image half) - the DMA engines do the 37MB of replication.
  - DVE: s = g * wg ([128,8192] bf16, 2x mode); TensorE contracts
    channels AND reduces the 4 corners via 4 strided-rhs matmuls per
    512-px block with a block-diagonal [128,128] lhsT (both images in
    one matmul), accumulating over (k, j) into [128,2048] f32 PSUM.
"""

import sys

sys.path.insert(0, "/opt/trn_rl_repo")

import ml_dtypes
import numpy as np

import concourse.bacc as bacc
import concourse.mybir as mybir
import concourse.tile as tile

F32 = mybir.dt.float32
BF16 = mybir.dt.bfloat16
I16 = mybir.dt.int16
I32 = mybir.dt.int32

N, CIN, COUT = 16, 64, 64
HIN, WIN = 66, 66
KH, KW = 3, 3
HO, WO = 64, 64
K = KH * KW
NPX = HO * WO  # 4096
NCORES = 8

TE = 68
NE = TE * TE  # 4624
CH = 2048  # px per iteration chunk
NIT = K * (NPX // CH)  # 18
COLS = NIT * 128  # 2304 idx-pipe columns
WC = K * NPX * 2 // 128  # 576 weight cells per partition
NCELL = 2 * K * NPX  # 73728 total weight cells


def _alu(name):
    return getattr(mybir.AluOpType, name)


def build_bass():
    nc = bacc.Bacc("TRN2", target_bir_lowering=False, debug=False,
                   num_devices=NCORES)

    din = {}
    for nm, shp in [
        ("img2", [128, HIN * WIN]),
        ("byc", [128, COLS]), ("bxc", [128, COLS]),
        ("dyc", [128, COLS]), ("dxc", [128, COLS]),
        ("byt", [128, WC]), ("bxt", [128, WC]),
        ("dyt", [128, WC]), ("dxt", [128, WC]),
        ("wk", [128, K * 128]),
    ]:
        din[nm] = nc.dram_tensor(nm, shp, BF16, kind="ExternalInput")
    out_d = nc.dram_tensor("out128", [128, NPX], F32, kind="ExternalOutput")
    wstage = nc.dram_tensor("wstage", [128, WC * 4], BF16)

    with tile.TileContext(nc) as tc:
        with tc.tile_pool(name="cst", bufs=1) as cpool:
            tab = cpool.tile([128, NE * 4], BF16, name="tab")
            vidx = cpool.tile([128, COLS], I16, name="vidx")
            wkt = cpool.tile([128, K * 128], BF16, name="wkt")

            t4 = tab[:].rearrange("p (ty tx d) -> p ty tx d", tx=TE, d=4)

            # ---------------- prologue ----------------
            with tc.tile_pool(name="pro", bufs=1) as pp:
                img = pp.tile([128, HIN * WIN], BF16, name="img")
                nc.sync.dma_start(img[:], din["img2"].ap())
                nc.sync.dma_start(wkt[:], din["wk"].ap())
                gt = {}
                for nm in ["byc", "bxc", "dyc", "dxc"]:
                    gt[nm] = pp.tile([128, COLS], BF16, name=nm)
                    nc.sync.dma_start(gt[nm][:], din[nm].ap())
                for nm in ["byt", "bxt", "dyt", "dxt"]:
                    gt[nm] = pp.tile([128, WC], BF16, name=nm)
                    nc.sync.dma_start(gt[nm][:], din[nm].ap())

                # table: zero borders then 4 shifted copies
                nc.gpsimd.memset(t4[:, 0:1, :, :], 0.0)
                nc.gpsimd.memset(t4[:, 66:68, :, :], 0.0)
                nc.gpsimd.memset(t4[:, :, 0:1, :], 0.0)
                nc.gpsimd.memset(t4[:, :, 66:68, :], 0.0)
                imgv = img[:].rearrange("p (h w) -> p h w", w=WIN)
                for jy in range(2):
                    for jx in range(2):
                        j = 2 * jy + jx
                        nc.scalar.copy(
                            t4[:, 1 - jy:67 - jy, 1 - jx:67 - jx, j],
                            imgv[:, :, :])

                # ---- idx pipeline ([128, COLS]) ----
                pyb = pp.tile([128, COLS], F32, name="pyb")
                nc.vector.tensor_add(pyb[:], gt["byc"][:], gt["dyc"][:])
                tyi = pp.tile([128, COLS], I32, name="tyi")
                nc.vector.tensor_copy(tyi[:], pyb[:])
                tyf = pp.tile([128, COLS], F32, name="tyf")
                nc.vector.tensor_copy(tyf[:], tyi[:])
                cty = pp.tile([128, COLS], F32, name="cty")
                nc.vector.tensor_scalar(cty[:], tyf[:], 0.0, 67.0,
                                        _alu("max"), _alu("min"))
                cty68 = pp.tile([128, COLS], F32, name="cty68")
                nc.vector.tensor_scalar(cty68[:], cty[:], float(TE), None,
                                        _alu("mult"))
                pxb = pp.tile([128, COLS], F32, name="pxb")
                nc.vector.tensor_add(pxb[:], gt["bxc"][:], gt["dxc"][:])
                txi = pp.tile([128, COLS], I32, name="txi")
                nc.vector.tensor_copy(txi[:], pxb[:])
                txf = pp.tile([128, COLS], F32, name="txf")
                nc.vector.tensor_copy(txf[:], txi[:])
                ctx = pp.tile([128, COLS], F32, name="ctx")
                nc.vector.tensor_scalar(ctx[:], txf[:], 0.0, 67.0,
                                        _alu("max"), _alu("min"))
                idxf = pp.tile([128, COLS], F32, name="idxf")
                nc.vector.tensor_add(idxf[:], cty68[:], ctx[:])
                nc.vector.tensor_copy(vidx[:], idxf[:])

                # ---- weight pipeline ([128, WC] cells) ----
                def axis_factors(bn, dn, f0n, f1n):
                    pb = pp.tile([128, WC], F32, name=f"pb_{bn}")
                    nc.vector.tensor_add(pb[:], gt[bn][:], gt[dn][:])
                    ti = pp.tile([128, WC], I32, name=f"ti_{bn}")
                    nc.vector.tensor_copy(ti[:], pb[:])
                    tf = pp.tile([128, WC], F32, name=f"tf_{bn}")
                    nc.vector.tensor_copy(tf[:], ti[:])
                    sub = pp.tile([128, WC], F32, name=f"sub_{bn}")
                    nc.vector.tensor_sub(sub[:], pb[:], tf[:])
                    mk = pp.tile([128, WC], BF16, name=f"mk_{bn}")
                    nc.vector.tensor_scalar(mk[:], tf[:], 0.0, None,
                                            _alu("is_ge"))
                    t0 = pp.tile([128, WC], BF16, name=f"t0_{bn}")
                    nc.vector.tensor_scalar(t0[:], sub[:], -1.0, 0.5,
                                            _alu("mult"), _alu("add"))
                    t1 = pp.tile([128, WC], BF16, name=f"t1_{bn}")
                    nc.vector.tensor_scalar(t1[:], sub[:], 0.5, None,
                                            _alu("add"))
                    f0 = pp.tile([128, WC], BF16, name=f0n)
                    nc.vector.tensor_mul(f0[:], t0[:], mk[:])
                    f1 = pp.tile([128, WC], BF16, name=f1n)
                    nc.vector.tensor_mul(f1[:], t1[:], mk[:])
                    return f0, f1

                f0y, f1y = axis_factors("byt", "dyt", "f0y", "f1y")
                f0x, f1x = axis_factors("bxt", "dxt", "f0x", "f1x")
                fy = [f0y, f1y]
                fx = [f0x, f1x]
                prod = pp.tile([128, WC * 4], BF16, name="prod")
                prodv = prod[:].rearrange("p (n j) -> p n j", j=4)
                for jy in range(2):
                    for jx in range(2):
                        nc.gpsimd.tensor_mul(prodv[:, :, 2 * jy + jx],
                                             fy[jy][:], fx[jx][:])
                nc.sync.dma_start(wstage.ap(), prod[:])

            # ---------------- main loop ----------------
            wlin = wstage.ap().rearrange("p n -> (p n)")
            with tc.tile_pool(name="gp", bufs=2) as gpool, \
                 tc.tile_pool(name="wp", bufs=2) as wgpool, \
                 tc.tile_pool(name="sp", bufs=2) as spool, \
                 tc.tile_pool(name="op", bufs=2) as opool, \
                 tc.tile_pool(name="ps", bufs=1, space="PSUM") as pspool:
                acc = [pspool.tile([128, CH], F32, name=f"acc{c}")
                       for c in range(2)]
                for k in range(K):
                    for c in range(2):
                        it = k * 2 + c
                        g = gpool.tile([128, CH * 4], BF16, tag="g",
                                       name=f"g_{it}")
                        gv = g[:].rearrange("p (n d) -> p n d", d=4)
                        nc.gpsimd.ap_gather(
                            gv, t4.rearrange("p ty tx d -> p (ty tx) d"),
                            vidx[:, it * 128:(it + 1) * 128],
                            channels=128, num_elems=NE, d=4, num_idxs=CH)
                        wg = wgpool.tile([128, CH * 4], BF16, tag="wg",
                                         name=f"wg_{it}")
                        for im in range(2):
                            start = (im * K * NPX + k * NPX + c * CH) * 4
                            src = wlin[start:start + CH * 4] \
                                .unsqueeze(0).broadcast_to([64, CH * 4])
                            nc.sync.dma_start(wg[im * 64:(im + 1) * 64, :],
                                              src)
                        s = spool.tile([128, CH * 4], BF16, tag="s",
                                       name=f"s_{it}")
                        nc.vector.tensor_mul(s[:], g[:], wg[:])
                        sv = s[:].rearrange("p (n d) -> p n d", d=4)
                        for blk in range(4):
                            for j in range(4):
                                nc.tensor.matmul(
                                    acc[c][:, blk * 512:(blk + 1) * 512],
                                    wkt[:, k * 128:(k + 1) * 128],
                                    sv[:, blk * 512:(blk + 1) * 512, j],
                                    start=(k == 0 and j == 0),
                                    stop=(k == K - 1 and j == 3))
                for c in range(2):
                    ot = opool.tile([128, CH], F32, tag="ot",
                                    name=f"ot_{c}")
                    nc.scalar.copy(ot[:], acc[c][:])
                    nc.sync.dma_start(
                        out_d.ap()[:, c * CH:(c + 1) * CH], ot[:])

    nc.compile()
    return nc


# ---------------- host side ----------------

def _host_arrays(input, offset, weight):
    bf = ml_dtypes.bfloat16
    inp = np.ascontiguousarray(input, dtype=np.float32)
    off = np.ascontiguousarray(offset, dtype=np.float32)
    w = np.ascontiguousarray(weight, dtype=np.float32)

    # block-diagonal lhsT per tap: wk[p=(im,ci), k*128 + (im,o)] = w[o,ci,k]
    w9 = w.reshape(COUT, CIN, K)  # [o, c, k]
    blk = w9.transpose(1, 2, 0)  # [c, k, o]
    wk = np.zeros((128, K, 128), np.float32)
    for im in range(2):
        wk[im * 64:(im + 1) * 64, :, im * 64:(im + 1) * 64] = blk
    wk = wk.reshape(128, K * 128).astype(bf)

    p = np.arange(128)
    im_p = p // 64

    # idx-pipe mapping: col = it*128 + cc; px = (it%2)*CH + cc*16 + p%16
    it = np.arange(NIT)
    cc = np.arange(128)
    k_it = it // 2  # [NIT]
    pxc = (it[:, None] % 2) * CH + cc[None, :] * 16  # [NIT, 128] (+ p%16)
    pxc = pxc[None, :, :] + (p % 16)[:, None, None]  # [128, NIT, 128]
    hoc = (pxc // WO).astype(np.float32)
    woc = (pxc % WO).astype(np.float32)
    khc = (k_it // KW).astype(np.float32)[None, :, None]
    kwc = (k_it % KW).astype(np.float32)[None, :, None]
    byc = (hoc + khc + 0.5).reshape(128, COLS).astype(bf)
    bxc = (woc + kwc + 0.5).reshape(128, COLS).astype(bf)

    # weight-pipe mapping: cell = p*WC + col = im*36864 + k*4096 + px
    cells = (p[:, None] * WC + np.arange(WC)[None, :])  # [128, WC]
    im_t = cells // (K * NPX)
    k_t = (cells % (K * NPX)) // NPX
    px_t = cells % NPX
    byt = ((px_t // WO) + (k_t // KW) + 0.5).astype(bf)
    bxt = ((px_t % WO) + (k_t % KW) + 0.5).astype(bf)

    offr = off.reshape(N, K, 2, NPX)

    in_maps = []
    for core in range(NCORES):
        na, nb = 2 * core, 2 * core + 1
        img2 = np.empty((128, HIN * WIN), np.float32)
        img2[0:64] = inp[na].reshape(64, -1)
        img2[64:128] = inp[nb].reshape(64, -1)

        dy2 = offr[[na, nb]][:, :, 0, :]  # [2, K, NPX]
        dx2 = offr[[na, nb]][:, :, 1, :]
        dyc = dy2[im_p[:, None, None], k_it[None, :, None],
                  pxc].reshape(128, COLS)
        dxc = dx2[im_p[:, None, None], k_it[None, :, None],
                  pxc].reshape(128, COLS)
        dyt = dy2[im_t, k_t, px_t]
        dxt = dx2[im_t, k_t, px_t]

        in_maps.append(dict(
            img2=img2.astype(bf),
            byc=byc, bxc=bxc,
            dyc=dyc.astype(bf), dxc=dxc.astype(bf),
            byt=byt, bxt=bxt,
            dyt=dyt.astype(bf), dxt=dxt.astype(bf),
            wk=wk,
        ))
    return in_maps


_NC_CACHE = None


def get_nc():
    global _NC_CACHE
    if _NC_CACHE is None:
        _NC_CACHE = build_bass()
    return _NC_CACHE


def kernel(input, offset, weight, _trace=False):
    from concourse.bass_utils import run_bass_kernel_spmd

    nc = get_nc()
    in_maps = _host_arrays(np.asarray(input), np.asarray(offset),
                           np.asarray(weight))
    res = run_bass_kernel_spmd(nc, in_maps, list(range(NCORES)), trace=_trace)
    out = np.empty((N, COUT, HO, WO), np.float32)
    for core in range(NCORES):
        o128 = np.asarray(res.results[core]["out128"])
        out[2 * core] = o128[0:64].reshape(COUT, HO, WO)
        out[2 * core + 1] = o128[64:128].reshape(COUT, HO, WO)
    if _trace:
        return out, res
    return out
